# revision 8
# baseline (speedup 1.0000x reference)
"""Trainium2 Bass kernel v2: batched single-head attention.

Reference computation (per batch b):
    q = x @ Wq + bq ; k = x @ Wk + bk ; v = x @ Wv + bv      # [S, H]
    out = softmax((q k^T) / sqrt(H)) @ v                     # [S, H]

Shapes: B=4, S=4096, D_IN=512, D_H=64. Sharding: 8 cores = (batch,
query-half); host rotates x[b] so core queries are rows 0:2048, and
pre-transposes to x^T [512, 4096] bf16.

v2 changes vs v1 (validated on HW; ~1.4-1.8x faster):
  - AV in NATURAL layout: out[q,h] accumulated as 8 matmuls per job with
    stationary P^T tiles [128,128] and 65-col V streams (HW microbench
    confirmed stationary weight loads are hidden even under 65-col
    streams) -> halves AV's PE stream time vs the out^T formulation.
    NOTE: the 4 accumulation groups sharing a PSUM bank require memset +
    start=False accumulation (matmul start=True resets the WHOLE bank).
  - exp split per job: ACT does cols 0:xa, DVE does the rest via the
    2-pass (1+z/1024)^1024 pow32 op. Per-phase split: h0 runs FULLY on
    ACT (the h0 DVE already carries the projection-extra evictions);
    h1 uses XA=744 -> neither engine paces alone.
  - K^T|V^T evicted + biased in ONE DVE op per chunk (combined [bk;bv]
    per-partition scalar; hw GPSIMD cannot read PSUM).
  - V transposed from bf16 kvt rows 64:128 (identity at base partition
    64 = bottom-right eye quadrant; 1 cyc/row).
  - S^T stream: 3 PSUM slots, lookahead-2, emission before extras;
    projection extras spread 2+2 matmuls across iterations.
  - head: DMA order w -> x chunk0 -> consts -> chunks; 5 junk warms.
  - tail: last TWO jobs full-ACT exp (keeps DVE pass2 latency out of
    the in-order PE queue ahead of the final AVs); piece A ships via
    DVE copy + SP queue mid-final-job, piece B via ACT copy + ACT
    queue (one HWDGE gen per queue, parallel).

Per-core dataflow:
  kvt[128,s]  = [Wk|Wv]^T x^T + [bk;bv]      (PE->psum, DVE evict)
  Q^T[64,2048] = Wq^T x^T + bq               (q cols only)
  vnat[128,kt,65] = PE-transpose of kvt V rows; col 64 = ones
  per job (kt, h):  S^T[128,1024] = K^T_kt^T Q^T_h      (PE)
                    P^T = exp(S^T/8): ACT cols 0:XA, DVE pow32 rest
                    out[q,65] += P^T_j^T V_ext (8 subtile matmuls, PE)
  out row q: col 64 = softmax denominator; host divides + reshapes.
"""

import os

import numpy as np


def _register_exp_op():
    """Custom DVE op: (c0*x + c1)^32 (affine + 5 squarings). Two chained
    passes give (1 + z/1024)^1024 ~ exp(z) at <=1.2e-3 rel err for |z|<3."""
    from concourse import dve_ops
    from concourse.dve_ops import DveOp
    from concourse.dve_spec import Spec, Src0, C0, C1, sq

    name = "EXP_POW32_ANT"
    if name in dve_ops._SUB_OPCODE_FOR_NAME:
        return next(o for o in dve_ops.OPS if o.name == name)

    def _ref(in0, in1, c0, c1, c2):
        u = in0.astype(np.float32) * np.float32(c0) + np.float32(c1)
        for _ in range(5):
            u = (u * u).astype(np.float32)
        return u

    op = DveOp(name, Spec(body=sq(sq(sq(sq(sq(Src0 * C0 + C1))))),
                          reference=_ref),
               subdim=False,
               uops_sha={"v3": "eafb894a1d5c531b",
                         "v4": "305ddd2af0946706"})
    row = dve_ops._CUSTOM_DVE_ROW_BASE + len(dve_ops.OPS)
    assert row < 0x20
    dve_ops.OPS.append(op)
    dve_ops.CUSTOM_DVE_SPECS[name] = op.spec
    dve_ops._SUB_OPCODE_FOR_NAME[name] = row
    return op


B, S, D_IN, D_H = 4, 4096, 512, 64
QW = S // 2          # queries per core
N_CORES = 8
NKT = S // 128       # 32 key tiles
NSC = S // 512       # 8 s chunks of 512
NDT = D_IN // 128    # 4 contraction tiles
HW = 1024            # attention job width (q cols per (kt,h) job)
XA = int(os.environ.get("K2_XA", "744"))  # exp cols on ACT; rest DVE pow32
XA_H0 = int(os.environ.get("K2_XA_H0", "1024"))  # h0 split (DVE busier there)
N_JUNK = 5           # PE p-state warm matmuls
CW = 66              # consts: id16(64 packed cols)|bkv|bq
OUT_SHAPE = (QW, 65)


def build_nc(repeats=1):
    """Build + compile the Bacc module for one core (SPMD across 8)."""
    import concourse.bass as bass
    import concourse.tile as tile
    from concourse import bacc, mybir

    f32 = mybir.dt.float32
    bf16 = mybir.dt.bfloat16
    EXP = mybir.ActivationFunctionType.Exp

    EXPOP = _register_exp_op()
    EXPC0 = 0.125 / 1024.0

    nc = bacc.Bacc("TRN2", target_bir_lowering=False, debug=False,
                   num_devices=N_CORES)

    xT_d = nc.dram_tensor("xT", (D_IN, S), bf16, kind="ExternalInput").ap()
    w_d = nc.dram_tensor("w", (D_IN, 192), bf16, kind="ExternalInput").ap()
    cst_d = nc.dram_tensor("consts", (128, CW), f32,
                           kind="ExternalInput").ap()
    y_dram = nc.dram_tensor("yT", OUT_SHAPE, f32, kind="ExternalOutput").ap()

    with tile.TileContext(nc) as tc:
        import contextlib
        with contextlib.ExitStack() as ctx:
            sb = ctx.enter_context(tc.tile_pool(name="sb", bufs=1))
            ptp = ctx.enter_context(tc.tile_pool(name="ptp", bufs=1))

            # ---- persistent buffers ----
            w_sb = sb.tile([128, NDT, 192], bf16)      # [Wk|Wv|Wq] d-tiles
            cst_sb = sb.tile([128, CW], f32)
            xt = sb.tile([128, NDT, S], bf16)          # x^T tiles
            kvt = sb.tile([128, S], bf16)              # rows 0:64 K^T, 64:128 V^T
            qt_sb = sb.tile([64, QW], bf16)            # Q^T
            vnat = sb.tile([128, NKT, 65], bf16)       # V natural + ones col
            y_sb = sb.tile([128, 16, 65], f32)         # out staging
            warm_sb = sb.tile([128, 4], f32)
            junk_sb = sb.tile([128, 512], bf16)

            bkv_sb = cst_sb[:, 64:65]                  # [bk;bv] per-partition
            bq_sb = cst_sb[0:D_H, 65:66]
            id16 = cst_sb[:, 0:64].bitcast(bf16)       # [128,128] bf16 eye
            y_r = y_dram.rearrange("(t p) h -> p t h", p=128)

            for _rep in range(repeats):
              with tc.tile_pool(name=f"pa{_rep}", bufs=1, space="PSUM") as pa:
                # input DMAs in need order: w + consts first (small), then
                # x^T chunks 0..7
                xT_r = xT_d.rearrange("(t p) s -> p t s", p=128)
                nc.sync.dma_start(w_sb, w_d.rearrange("(t p) m -> p t m",
                                                      p=128))
                nc.sync.dma_start(xt[:, :, 0:512], xT_r[:, :, 0:512])
                nc.sync.dma_start(cst_sb, cst_d)
                for c in range(1, NSC):
                    cs = slice(512 * c, 512 * (c + 1))
                    nc.sync.dma_start(xt[:, :, cs], xT_r[:, :, cs])

                # seed junk/warm tiles via the idle Pool engine (junk first
                # so PE p-state warms can begin at t~0)
                nc.gpsimd.memset(junk_sb, 1.0)
                nc.gpsimd.memset(warm_sb, 1.0)
                # warm-ups: pre-touch engines one semaphore at a time
                nc.scalar.activation(warm_sb[0:1, 2:3], warm_sb[0:1, 3:4],
                                     EXP, scale=1.0)
                nc.vector.tensor_copy(warm_sb[:, 0:1], bkv_sb)
                nc.vector.memset(vnat[:, :, 64:65], 1.0)
                warm = pa.tile([128, HW], f32, tag="st", bufs=3)
                for _ in range(N_JUNK):
                    nc.tensor.matmul(warm[:, 0:512], lhsT=junk_sb[:, 0:128],
                                     rhs=junk_sb, start=True, stop=True)

                def proj_kv_mm(c, dts, pkv=None):
                    # K^T and V^T projected together (P-packed matmul); the
                    # accumulation group may stay open across iterations
                    # (interleaved AVs are start=False, S^Ts hit other banks)
                    cs = slice(512 * c, 512 * (c + 1))
                    if pkv is None:
                        pkv = pa.tile([128, 640], f32, tag="st", name="pkv",
                                      bufs=3)
                    for dt in dts:
                        nc.tensor.matmul(
                            pkv[:, 0:512],
                            lhsT=w_sb[:, dt, 0:128], rhs=xt[:, dt, cs],
                            start=(dt == 0), stop=(dt == NDT - 1))
                    if dts[-1] == NDT - 1:
                        # evicted + biased in ONE DVE op (hw GPSIMD cannot
                        # read PSUM)
                        nc.vector.tensor_scalar_add(kvt[:, cs],
                                                    pkv[:, 0:512], bkv_sb)
                    return pkv

                def proj_kv(c):
                    return proj_kv_mm(c, range(NDT))

                def v_tr(c, t):
                    # 4 PE transposes of V^T key tiles (bf16 in kvt, 1
                    # cyc/row) into t's tail bytes, DVE evicts to vnat
                    tb = t[:, 512:512 + 2 * D_H].bitcast(bf16)
                    for i in range(4):
                        kt = 4 * c + i
                        nc.tensor.transpose(
                            tb[:, D_H * i:D_H * (i + 1)],
                            in_=kvt[64:128, 128 * kt:128 * (kt + 1)],
                            identity=id16[64:128, 64:128])
                    nc.vector.tensor_copy(
                        vnat[:, 4 * c:4 * (c + 1), 0:D_H],
                        tb.rearrange("p (t h) -> p t h", h=D_H))

                def proj_q_mm(c, w_, dts, pq=None):
                    # q chunk of width w_; like proj_kv_mm the accumulation
                    # group may stay open across iterations
                    cs = slice(512 * c, 512 * c + w_)
                    if pq is None:
                        pq = pa.tile([128, w_], f32, tag="st", name="pq",
                                     bufs=3)
                    for dt in dts:
                        nc.tensor.matmul(
                            pq[0:D_H, 0:w_],
                            lhsT=w_sb[:, dt, 128:192], rhs=xt[:, dt, cs],
                            start=(dt == 0), stop=(dt == NDT - 1))
                    if dts[-1] == NDT - 1:
                        nc.vector.tensor_scalar_add(
                            qt_sb[:, cs], pq[0:D_H, 0:w_], bq_sb)
                    return pq

                def proj_q(c):
                    return proj_q_mm(c, 512, range(NDT))

                # minimal head: only what S^T(kt0, h0) needs
                pkv0 = proj_kv(0)
                proj_q(0)
                proj_q(1)
                v_tr(0, pkv0)

                def st_tile(kt, h):
                    pst = pa.tile([128, HW], f32, tag="st", bufs=3,
                                  name=f"pst_{kt}_{h}")
                    for c in range(2):
                        cs = slice(512 * c, 512 * (c + 1))
                        qs = slice(HW * h + 512 * c, HW * h + 512 * (c + 1))
                        nc.tensor.matmul(
                            pst[:, cs],
                            lhsT=kvt[0:64, 128 * kt:128 * (kt + 1)],
                            rhs=qt_sb[:, qs],
                            start=True, stop=True)
                    return pst

                jobs = [(k, 0) for k in range(NKT)] + [(k, 1) for k in
                                                       range(NKT)]
                NJ = 2 * NKT

                # deadline-paced extras: chunk c's K^T needed when
                # S^T(4c, h0) is EMITTED (job 4c-3); V tiles by AV(4c);
                # q chunks 2-3 by the first h=1 S^T emission (job 29).
                # extras on a deadline schedule: chunk c's K^T needed by
                # st(4c) emission (iter 4c-2), its vnat tiles (via the SP
                # XBAR queue) by av(4c); q chunks 2-3 by st(32) (iter 30)
                pend = {}
                extra_at = {}

                def sched(j, fn):
                    extra_at.setdefault(j, []).append(fn)

                alloc_at = set()

                def sched_alloc(j, fn):
                    sched(j, fn)
                    alloc_at.add(j)

                sched_alloc(0, lambda: pend.__setitem__(1, proj_kv(1)))
                sched(2, lambda: v_tr(1, pend.pop(1)))
                for c in range(2, NSC):
                    sched_alloc(4 * c - 6, lambda c=c: pend.__setitem__(
                        c, proj_kv_mm(c, [0, 1])))
                    sched(4 * c - 5, lambda c=c: proj_kv_mm(
                        c, [2, 3], pend[c]))
                    sched(4 * c - 3, lambda c=c: v_tr(c, pend.pop(c)))
                sched_alloc(24, lambda: proj_q(2))
                sched_alloc(27, lambda: proj_q(3))

                pouts = {}  # (h, piece) -> psum tile [128, 260]

                def out_tile(h):
                    # 4 interleaved accumulation groups share each bank, and
                    # matmul start=True resets the WHOLE bank -> zero the
                    # bank once and accumulate with start=False throughout
                    for p, tag in ((0, "oA"), (1, "oB")):
                        pouts[(h, p)] = pa.tile([128, 4, 65], f32, tag=tag,
                                                bufs=1, name=f"o{h}{p}")
                        nc.vector.memset(pouts[(h, p)], 0.0)

                SHIP_CP = {"s": nc.scalar, "v": nc.vector}
                SHIP_DMA = {"sp": nc.sync, "act": nc.scalar}

                def av(kt, h, pt, subs):
                    for j in subs:
                        nc.tensor.matmul(
                            pouts[(h, j // 4)][:, j % 4, :],
                            lhsT=pt[:, 128 * j:128 * (j + 1)],
                            rhs=vnat[:, kt, :],
                            start=False, stop=(kt == NKT - 1),
                            skip_group_check=True)

                def ship(h, p, lo=0, hi=4, cp="v", dq="sp"):
                    ts = 8 * h + 4 * p + lo
                    n = hi - lo
                    if cp == "s":
                        nc.scalar.copy(y_sb[:, ts:ts + n, :],
                                       pouts[(h, p)][:, lo:hi, :])
                    else:
                        nc.vector.tensor_copy(y_sb[:, ts:ts + n, :],
                                              pouts[(h, p)][:, lo:hi, :])
                    SHIP_DMA[dq].dma_start(y_r[:, ts:ts + n, :],
                                           y_sb[:, ts:ts + n, :])

                out_tile(0)
                # job-0's S^T as TWO 1-bank tiles: the first needs only qt
                # chunk 0, so ACT's exp stream starts ~1-2us earlier than a
                # [128,1024] tile gated on chunk 1
                pst0 = []
                for c in range(2):
                    p0 = pa.tile([128, 512], f32, tag="st", bufs=3,
                                 name=f"pst0_{c}")
                    nc.tensor.matmul(
                        p0, lhsT=kvt[0:64, 0:128],
                        rhs=qt_sb[:, 512 * c:512 * (c + 1)],
                        start=True, stop=True)
                    pst0.append(p0)
                psts = {1: st_tile(*jobs[1])}
                # warm the matmul-weight-load and transpose paths
                nc.tensor.matmul(warm[:, 0:2], lhsT=w_sb[:, 0, 0:128],
                                 rhs=w_sb[:, 0, 0:2], start=True, stop=True)
                nc.tensor.transpose(warm[0:1, 4:68].bitcast(bf16),
                                    in_=id16[:, 0:1], identity=id16)

                NSUB = HW // 128          # 8 q-subtiles per job
                for j in range(NJ):
                    kt, h = jobs[j]
                    if h == 1 and (1, 0) not in pouts:
                        out_tile(1)
                    pst = None if j == 0 else psts.pop(j)
                    # per-phase split: h0's DVE also carries the extras'
                    # evictions, so ACT takes more exp columns there; last
                    # two jobs full-ACT (keeps DVE pass2 latency off the
                    # drain-critical tail)
                    xa = (HW if j >= NJ - 2 else
                          XA_H0 if h == 0 else XA)
                    sa = xa // 128
                    pt = ptp.tile([128, HW], bf16, tag="pt", bufs=6,
                                  name="ptile")
                    if j == 0:
                        for c in range(2):
                            cs = slice(512 * c, 512 * (c + 1))
                            nc.scalar.activation(pt[:, cs], pst0[c], EXP,
                                                 scale=0.125)
                    else:
                        nc.scalar.activation(pt[:, 0:xa], pst[:, 0:xa], EXP,
                                             scale=0.125)
                    if xa < HW and j > 0:
                        mid = ptp.tile([128, HW - xa], f32, tag="mid",
                                       bufs=3, name="mid")
                        nc.vector._custom_dve(EXPOP, out=mid,
                                              in0=pst[:, xa:HW],
                                              s0=EXPC0, s1=1.0)
                        nc.vector._custom_dve(EXPOP, out=pt[:, xa:HW],
                                              in0=mid, s0=1.0, s1=0.0)
                    # lookahead-2 S^T BEFORE extras: with 3 st slots the
                    # emission reuses pst(j-1)'s slot (already read), so
                    # extras rarely stall the ACT exp stream
                    if j + 2 < NJ:
                        psts[j + 2] = st_tile(*jobs[j + 2])
                    for fn in extra_at.pop(j, ()):
                        fn()
                    if j == NJ - 1:
                        # tail: piece A ships between subtile groups;
                        # piece B splits over two copy engines + two DMA
                        # queues (parallel HWDGE gens)
                        av(kt, h, pt, range(4))
                        ship(1, 0, cp="v", dq="sp")
                        av(kt, h, pt, range(4, NSUB))
                        ship(1, 1, cp="s", dq="act")
                    else:
                        av(kt, h, pt, range(sa))
                        av(kt, h, pt, range(sa, NSUB))
                    if j == NKT - 1:
                        ship(0, 0, cp="v")
                        ship(0, 1, cp="s")
                assert not extra_at and not pend

    nc.compile()
    return nc


def _prep_core_inputs(c, x, Wq, bq, Wk, bk, Wv, bv):
    import ml_dtypes
    bf16 = ml_dtypes.bfloat16
    b, qh = c // 2, c % 2
    xb = x[b]
    if qh:
        xb = np.concatenate([xb[QW:], xb[:QW]], axis=0)
    consts = np.zeros((128, CW), np.float32)
    # cols 0:64: bf16 eye(128), packed 2-per-f32
    eye16 = np.eye(128, dtype=bf16).view(np.uint16).reshape(128, 64, 2)
    packed = (eye16[:, :, 0].astype(np.uint32)
              | (eye16[:, :, 1].astype(np.uint32) << 16))
    consts[:, 0:64] = packed.view(np.float32)
    consts[:, 64] = np.concatenate([bk, bv])   # [bk;bv] per-partition bias
    consts[0:D_H, 65] = bq
    return {
        "xT": np.ascontiguousarray(xb.T).astype(bf16),
        "w": np.ascontiguousarray(
            np.concatenate([Wk, Wv, Wq], axis=1)).astype(bf16),
        "consts": consts,
    }


def gather_output(per_core_y):
    """per_core_y: list of 8 arrays [QW, 65] -> full y [B, S, D_H]."""
    y = np.empty((B, S, D_H), np.float32)
    for c in range(N_CORES):
        b, qh = c // 2, c % 2
        yn = np.asarray(per_core_y[c])
        y[b, qh * QW:(qh + 1) * QW] = yn[:, 0:D_H] / yn[:, D_H:D_H + 1]
    return y


def gather_output_bv(per_core_y, bv):
    y = gather_output(per_core_y)
    y += np.asarray(bv, np.float32)
    return y


def run(x, Wq, bq, Wk, bk, Wv, bv, trace=False):
    """Returns (y [B,S,H], BassKernelResults)."""
    from concourse import bass_utils

    x = np.asarray(x, np.float32)
    in_maps = [
        _prep_core_inputs(c, x, np.asarray(Wq, np.float32),
                          np.asarray(bq, np.float32),
                          np.asarray(Wk, np.float32),
                          np.asarray(bk, np.float32),
                          np.asarray(Wv, np.float32),
                          np.asarray(bv, np.float32))
        for c in range(N_CORES)
    ]
    nc = build_nc()
    res = bass_utils.run_bass_kernel_spmd(
        nc, in_maps, core_ids=list(range(N_CORES)), trace=trace)
    y = gather_output([res.results[c]["yT"] for c in range(N_CORES)])
    return y, res


def kernel(x, Wq, bq, Wk, bk, Wv, bv):
    y, _ = run(x, Wq, bq, Wk, bk, Wv, bv, trace=False)
    return y
